# revision 1
# baseline (speedup 1.0000x reference)
"""Bahdanau attention kernel for Trainium2 (8 NeuronCores, data-parallel over batch).

Reference computation (per batch row b):
    pq      = query @ Wq.T                       # (B, AD)
    hidden  = tanh(pq[:, None, :] + processed_memory)   # (B, T, AD)
    e       = einsum('btd,d->bt', hidden, v)     # (B, T)
    e       = where(mask, -1e30, e)
    out     = softmax(e, axis=1)

Device strategy (per core, 8 batches):
  * processed_memory is host-transposed to [b, AD, T] so AD sits on SBUF
    partitions.  The per-d "+pq" add then folds into the ScalarE tanh as a
    per-partition activation bias (free), and the v-weighted reduction over d
    becomes TensorE matmuls with a [128,1] stationary v column (M=1, free up
    to 512) accumulating in PSUM.
  * Energies strips [1, 2048] leave PSUM via a VectorE copy, then tiny
    SBUF->SBUF DMAs relayout them into an [8, T] tile (one batch per
    partition) where the masked softmax runs along the free dimension:
    exp on ScalarE, mask-multiply + row-sum fused in one
    tensor_tensor_reduce, reciprocal + scale on VectorE.
  * mask is applied multiplicatively: softmax(where(m,-1e30,e)) ==
    exp(e)*(1-m) / sum(exp(e)*(1-m)) exactly (exp(-1e30) underflows to 0,
    and |e| <= sum|v| ~ 13 so exp(e) cannot overflow in fp32).
"""

import sys

if "/opt/trn_rl_repo" not in sys.path:
    sys.path.insert(0, "/opt/trn_rl_repo")

import numpy as np

import concourse.bacc as bacc
import concourse.bass as bass
import concourse.tile as tile
from concourse import mybir
from concourse.bass_utils import run_bass_kernel_spmd

B, T, QD, AD = 64, 4096, 1024, 256
NCORES = 8
BLOC = B // NCORES  # batches per core
KB = QD // 128      # k-blocks for the pq matmul
DB = AD // 128      # d-blocks (partition blocks of AD)
F32 = mybir.dt.float32
F16 = mybir.dt.float16
U8 = mybir.dt.uint8


def build_nc() -> bass.Bass:
    # Bacc (not plain Bass): its nop/event-semaphore lowering passes are what
    # let Tile-scheduled instructions carry multiple semaphore waits.
    nc = bacc.Bacc(None, target_bir_lowering=False)

    # fp16: halves the dominant HBM stream; pm ~ N(0,1) so fp16 quantization
    # (10 mantissa bits) costs ~2e-4 rel err on the softmax output
    pm_t = nc.declare_dram_parameter("pm_t", [BLOC, AD, T], F16, isOutput=False)
    # qT[p, kb*BLOC + b] = query[b, kb*128 + p]  (host-packed, partition-major)
    qT = nc.declare_dram_parameter("qT", [128, KB * BLOC], F32, isOutput=False)
    msk = nc.declare_dram_parameter("mask", [BLOC, T], U8, isOutput=False)
    WqT = nc.declare_dram_parameter("WqT", [QD, AD], F32, isOutput=False)
    v_r = nc.declare_dram_parameter("v_r", [128, DB], F32, isOutput=False)
    # block-indicator matrices for the softmax cross-partition matmuls:
    # sel16[p, b] = 1.0 iff p // 16 == b ; sel16T is its transpose
    sel16_d = nc.declare_dram_parameter("sel16", [128, B // NCORES], F32, isOutput=False)
    sel16T_d = nc.declare_dram_parameter("sel16T", [B // NCORES, 128], F32, isOutput=False)
    out = nc.declare_dram_parameter("out", [BLOC, T], F32, isOutput=True)

    Tanh = mybir.ActivationFunctionType.Tanh
    Exp = mybir.ActivationFunctionType.Exp
    mult = mybir.AluOpType.mult
    add = mybir.AluOpType.add

    HT = 2048          # energies strip length (4 PSUM banks)
    NMM = HT // 512    # matmuls per strip per d-block
    PB = 16            # partitions per batch in the softmax layout
    PF = T // PB       # 256 free elements per partition

    with tile.TileContext(nc) as tc:
        with (
            tc.tile_pool(name="singles", bufs=1) as singles,
            tc.tile_pool(name="pm", bufs=8) as pm_pool,
            tc.tile_pool(name="hid", bufs=6) as hid_pool,
            tc.tile_pool(name="estrip", bufs=4) as estrip_pool,
            tc.tile_pool(name="epsum", bufs=2, space="PSUM") as epsum_pool,
        ):
            # ---- constant loads (wq/qt first: they gate pq -> first tanh) ----
            wq_sb = singles.tile([128, KB, AD], F32)
            nc.sync.dma_start(
                out=wq_sb, in_=WqT[:, :].rearrange("(kb p) d -> p kb d", p=128)
            )
            qt_sb = singles.tile([128, KB, BLOC], F32)
            nc.sync.dma_start(
                out=qt_sb, in_=qT[:, :].rearrange("p (kb b) -> p kb b", b=BLOC)
            )
            v_sb = singles.tile([128, DB], F32)
            nc.sync.dma_start(out=v_sb, in_=v_r[:, :])
            # fp16 copy of v for the energies matmuls: fp32 matmuls run as
            # two PE passes at ~4x the cost; tanh outputs are in [-1,1] and
            # v is small, so fp16 (10 mantissa bits) costs ~3e-4 rel err.
            v16_sb = singles.tile([128, DB], F16)
            nc.vector.tensor_copy(out=v16_sb, in_=v_sb)

            # ---- pq = Wq @ query.T, laid out [d % 128, dblk, b] ----
            pq_sb = singles.tile([128, DB, BLOC], F32)
            for d in range(DB):
                ppq = epsum_pool.tile([128, BLOC], F32, tag="ep")
                for k in range(KB):
                    nc.tensor.matmul(
                        ppq,
                        lhsT=wq_sb[:, k, d * 128 : (d + 1) * 128],
                        rhs=qt_sb[:, k, :],
                        start=(k == 0),
                        stop=(k == KB - 1),
                    )
                nc.scalar.copy(pq_sb[:, d, :], ppq)

            e2_sb = singles.tile([128, PF], F32)
            work2 = singles.tile([128, PF], F32)
            colsum = singles.tile([128, 1], F32)
            rinv_sb = singles.tile([BLOC, 1], F32)

            # ---- main loop: tanh + v-reduction ----
            for b in range(BLOC):
                hid = []
                for d in range(DB):
                    pm_sb = pm_pool.tile([128, T], F16)
                    nc.sync.dma_start(
                        out=pm_sb, in_=pm_t[b, d * 128 : (d + 1) * 128, :]
                    )
                    h = hid_pool.tile([128, T], F16)
                    nc.scalar.activation(
                        out=h,
                        in_=pm_sb,
                        func=Tanh,
                        bias=pq_sb[:, d, b : b + 1],
                        scale=1.0,
                    )
                    hid.append(h)
                for half in range(T // HT):
                    ep = epsum_pool.tile([1, HT], F32, tag="ep")
                    for c in range(NMM):
                        lo = half * HT + c * 512
                        nc.tensor.matmul(
                            ep[:, c * 512 : (c + 1) * 512],
                            lhsT=v16_sb[:, 0:1],
                            rhs=hid[0][:, lo : lo + 512],
                            start=True,
                            stop=False,
                        )
                        nc.tensor.matmul(
                            ep[:, c * 512 : (c + 1) * 512],
                            lhsT=v16_sb[:, 1:2],
                            rhs=hid[1][:, lo : lo + 512],
                            start=False,
                            stop=True,
                        )
                    es = estrip_pool.tile([1, HT], F32)
                    nc.vector.tensor_copy(out=es, in_=ep)
                    p0 = b * PB + half * (HT // PF)
                    nc.gpsimd.dma_start(
                        out=e2_sb[p0 : p0 + HT // PF, :], in_=es
                    )

            # ---- softmax-side constants ----
            # energies layout for the post pass: partition p = b*PB + q holds
            # t in [ (p%PB)*PF, ... ) of batch b = p//PB -> all 128 partitions
            # work during the softmax instead of 8.
            mask2_sb = singles.tile([128, PF], U8)
            nc.sync.dma_start(
                out=mask2_sb, in_=msk[:, :].rearrange("b (q f) -> (b q) f", f=PF)
            )
            maskz2_sb = singles.tile([128, PF], F32)
            nc.vector.tensor_scalar(
                out=maskz2_sb,
                in0=mask2_sb,
                scalar1=-1.0,
                scalar2=1.0,
                op0=mult,
                op1=add,
            )
            sel16 = singles.tile([128, BLOC], F32)
            nc.sync.dma_start(out=sel16, in_=sel16_d[:, :])
            sel16T = singles.tile([BLOC, 128], F32)
            nc.sync.dma_start(out=sel16T, in_=sel16T_d[:, :])


            # ---- masked softmax, all 128 partitions busy ----
            nc.scalar.activation(out=work2, in_=e2_sb, func=Exp)
            # (tensor_tensor_reduce is a custom ant-dve ucode op that faults
            # on this runtime — use the two standard ops instead)
            nc.vector.tensor_mul(work2, work2, maskz2_sb)
            nc.vector.reduce_sum(out=colsum, in_=work2, axis=mybir.AxisListType.X)
            # per-batch row sums: rowsum[b] = sum_p sel16[p, b] * colsum[p]
            psum_rs = epsum_pool.tile([BLOC, 1], F32, tag="ep")
            nc.tensor.matmul(psum_rs, lhsT=sel16, rhs=colsum, start=True, stop=True)
            nc.vector.reciprocal(out=rinv_sb, in_=psum_rs)
            # broadcast 1/rowsum back to the 16 partitions of each batch
            psum_ri = epsum_pool.tile([128, 1], F32, tag="ep")
            nc.tensor.matmul(psum_ri, lhsT=sel16T, rhs=rinv_sb, start=True, stop=True)
            nc.vector.tensor_scalar_mul(out=work2, in0=work2, scalar1=psum_ri)
            nc.sync.dma_start(
                out=out[:, :].rearrange("b (q f) -> (b q) f", f=PF), in_=work2
            )

    # Run the Bacc lowering passes (move_matmul_waits_to_ldweights,
    # generate_event_semaphores, alloc_regs, ...) — run_bass_via_pjrt takes
    # the module as-is and walrus rejects unlowered multi-wait instructions.
    nc.finalize()
    return nc


_CACHE: dict = {}


def _get_nc() -> bass.Bass:
    if "nc" not in _CACHE:
        _CACHE["nc"] = build_nc()
    return _CACHE["nc"]


def make_in_maps(query, processed_memory, mask, Wq, v):
    query = np.ascontiguousarray(np.asarray(query, dtype=np.float32))
    pm = np.asarray(processed_memory, dtype=np.float32)
    mask_u8 = np.asarray(mask).astype(np.uint8)
    Wq = np.asarray(Wq, dtype=np.float32)
    v = np.asarray(v, dtype=np.float32)

    WqT = np.ascontiguousarray(Wq.T)                  # (QD, AD)
    v_r = np.ascontiguousarray(v.reshape(DB, 128).T)  # (128, DB)
    sel16 = np.zeros((128, BLOC), dtype=np.float32)
    for b in range(BLOC):
        sel16[b * 16 : (b + 1) * 16, b] = 1.0
    sel16T = np.ascontiguousarray(sel16.T)

    in_maps = []
    for i in range(NCORES):
        sl = slice(i * BLOC, (i + 1) * BLOC)
        in_maps.append(
            {
                "pm_t": np.ascontiguousarray(
                    pm[sl].transpose(0, 2, 1).astype(np.float16)
                ),
                "qT": np.ascontiguousarray(
                    query[sl]
                    .T.reshape(KB, 128, BLOC)
                    .transpose(1, 0, 2)
                    .reshape(128, KB * BLOC)
                ),
                "mask": np.ascontiguousarray(mask_u8[sl]),
                "WqT": WqT,
                "v_r": v_r,
                "sel16": sel16,
                "sel16T": sel16T,
            }
        )
    return in_maps


def run_spmd(in_maps, **kwargs):
    return run_bass_kernel_spmd(_get_nc(), in_maps, list(range(NCORES)), **kwargs)


def kernel(query, processed_memory, mask, Wq, v) -> np.ndarray:
    in_maps = make_in_maps(query, processed_memory, mask, Wq, v)
    res = run_spmd(in_maps)
    return np.concatenate(
        [res.results[i]["out"] for i in range(NCORES)], axis=0
    ).astype(np.float32)



# revision 2
# speedup vs baseline: 1.7575x; 1.7575x over previous
"""Bahdanau attention kernel for Trainium2 (8 NeuronCores, data-parallel over batch).

Reference computation (per batch row b):
    pq      = query @ Wq.T                       # (B, AD)
    hidden  = tanh(pq[:, None, :] + processed_memory)   # (B, T, AD)
    e       = einsum('btd,d->bt', hidden, v)     # (B, T)
    e       = where(mask, -1e30, e)
    out     = softmax(e, axis=1)

Key optimization: ~50% of positions are masked and their energies are
discarded (softmax weight exactly 0 since exp(-1e30) underflows).  The host
shard step compacts each batch row to its unmasked columns (padded to NT),
so the device streams/computes only the surviving half of processed_memory.
The host scatters device probabilities back to the full (B, T) grid with
zeros at masked positions -- bit-identical to the reference's where().

Device strategy (per core, 8 batches):
  * pm compacted+transposed to [b, 128, (db, NT)] fp16 so AD sits on SBUF
    partitions.  The per-d "+pq" add folds into the ScalarE tanh as a
    per-partition activation bias; tanh output hid is fp16.
  * Energies land DIRECTLY in the softmax layout [128, PF] (partition
    p = b*16 + q holds positions q*PF..q*PF+PF of batch b): for each
    (b, q, db) a matmul with a sliding-window one-hot stationary
    ZV[:, db, 127-p : 255-p] (v at column p, zeros elsewhere) accumulates
    v . hid[:, q window] into PSUM row p, adding zeros to all other rows.
    One PSUM tile, 256 accumulating matmuls, no PSUM->SBUF relayout at all.
  * Masked softmax runs on all 128 partitions: exp straight out of PSUM on
    ScalarE, multiply by a validity mask (kills padding), free-dim
    reduce_sum, cross-partition row sums via a tiny sel16 matmul,
    reciprocal, broadcast back via matmul, scale, DMA out.
"""

import sys

if "/opt/trn_rl_repo" not in sys.path:
    sys.path.insert(0, "/opt/trn_rl_repo")

import numpy as np

import concourse.bacc as bacc
import concourse.bass as bass
import concourse.tile as tile
from concourse import mybir
from concourse.bass_utils import run_bass_kernel_spmd

B, T, QD, AD = 64, 4096, 1024, 256
NCORES = 8
BLOC = B // NCORES  # batches per core
KB = QD // 128      # k-blocks for the pq matmul
DB = AD // 128      # d-blocks (partition blocks of AD)
NT_DEFAULT = 2176   # device column capacity per row (>= max unmasked count)
F32 = mybir.dt.float32
F16 = mybir.dt.float16


def build_nc(NT: int) -> bass.Bass:
    assert NT % 128 == 0
    PF = NT // 16        # free elems per partition in the softmax layout
    NQ = 16              # partition groups per batch

    # Bacc (not plain Bass): its nop/event-semaphore lowering passes are what
    # let Tile-scheduled instructions carry multiple semaphore waits.
    nc = bacc.Bacc(None, target_bir_lowering=False)

    pm_c = nc.declare_dram_parameter("pm_c", [BLOC, 128, DB, NT], F16, isOutput=False)
    # qT[p, kb*BLOC + b] = query[b, kb*128 + p]  (host-packed, partition-major)
    qT = nc.declare_dram_parameter("qT", [128, KB * BLOC], F16, isOutput=False)
    WqT = nc.declare_dram_parameter("WqT", [QD, AD], F16, isOutput=False)
    # sliding-window one-hot v: ZV[p, db, j] = v[db*128+p] iff j == 127
    zv_d = nc.declare_dram_parameter("zv", [128, DB, 255], F16, isOutput=False)
    valid_d = nc.declare_dram_parameter("valid2", [128, PF], F32, isOutput=False)
    sel16_d = nc.declare_dram_parameter("sel16", [128, BLOC], F32, isOutput=False)
    sel16T_d = nc.declare_dram_parameter("sel16T", [BLOC, 128], F32, isOutput=False)
    out = nc.declare_dram_parameter("out", [BLOC, NT], F32, isOutput=True)

    Tanh = mybir.ActivationFunctionType.Tanh
    Exp = mybir.ActivationFunctionType.Exp

    with tile.TileContext(nc) as tc:
        with (
            tc.tile_pool(name="singles", bufs=1) as singles,
            tc.tile_pool(name="pm", bufs=5) as pm_pool,
            tc.tile_pool(name="hid", bufs=3) as hid_pool,
            tc.tile_pool(name="psum_pq", bufs=2, space="PSUM") as psum_pq,
            tc.tile_pool(name="psum_e", bufs=1, space="PSUM") as psum_e,
        ):
            # ---- constant loads (qt/wq first: they gate pq -> first tanh) ----
            qt_sb = singles.tile([128, KB, BLOC], F16)
            nc.sync.dma_start(
                out=qt_sb, in_=qT[:, :].rearrange("p (kb b) -> p kb b", b=BLOC)
            )
            wq_sb = []
            for db in range(DB):
                w = singles.tile([128, KB, 128], F16)
                nc.sync.dma_start(
                    out=w,
                    in_=WqT[:, db * 128 : (db + 1) * 128].rearrange(
                        "(kb p) d -> p kb d", p=128
                    ),
                )
                wq_sb.append(w)
            zv_sb = singles.tile([128, DB, 255], F16)
            nc.sync.dma_start(out=zv_sb, in_=zv_d[:, :, :])
            valid_sb = singles.tile([128, PF], F32)
            nc.sync.dma_start(out=valid_sb, in_=valid_d[:, :])
            sel16 = singles.tile([128, BLOC], F32)
            nc.sync.dma_start(out=sel16, in_=sel16_d[:, :])
            sel16T = singles.tile([BLOC, 128], F32)
            nc.sync.dma_start(out=sel16T, in_=sel16T_d[:, :])

            # ---- pq = Wq @ query.T, laid out [d % 128, dblk, b] ----
            pq_sb = singles.tile([128, DB, BLOC], F32)
            for db in range(DB):
                ppq = psum_pq.tile([128, BLOC], F32, tag="pq")
                for k in range(KB):
                    nc.tensor.matmul(
                        ppq,
                        lhsT=wq_sb[db][:, k, :],
                        rhs=qt_sb[:, k, :],
                        start=(k == 0),
                        stop=(k == KB - 1),
                    )
                nc.vector.tensor_copy(out=pq_sb[:, db, :], in_=ppq)

            # ---- energies accumulator in the softmax layout ----
            # ep[b*16 + q, f] = sum_d v_d * tanh(pq + pm)[b, q*PF + f]
            ep = psum_e.tile([128, PF], F32, tag="e")

            # ---- main loop: tanh + windowed v-reduction straight to layout ----
            for b in range(BLOC):
                pm_sb = pm_pool.tile([128, DB, NT], F16)
                nc.sync.dma_start(out=pm_sb, in_=pm_c[b])
                hid = hid_pool.tile([128, DB, NT], F16)
                for db in range(DB):
                    nc.scalar.activation(
                        out=hid[:, db, :],
                        in_=pm_sb[:, db, :],
                        func=Tanh,
                        bias=pq_sb[:, db, b : b + 1],
                        scale=1.0,
                    )
                for q in range(NQ):
                    p = b * NQ + q
                    for db in range(DB):
                        nc.tensor.matmul(
                            ep,
                            lhsT=zv_sb[:, db, 127 - p : 255 - p],
                            rhs=hid[:, db, q * PF : (q + 1) * PF],
                            start=(b == 0 and q == 0 and db == 0),
                            stop=(b == BLOC - 1 and q == NQ - 1 and db == DB - 1),
                        )

            # ---- masked softmax, all 128 partitions busy ----
            work2 = singles.tile([128, PF], F32)
            colsum = singles.tile([128, 1], F32)
            rinv_sb = singles.tile([BLOC, 1], F32)
            nc.scalar.activation(out=work2, in_=ep, func=Exp)
            nc.vector.tensor_mul(work2, work2, valid_sb)
            nc.vector.reduce_sum(out=colsum, in_=work2, axis=mybir.AxisListType.X)
            # per-batch row sums: rowsum[b] = sum_p sel16[p, b] * colsum[p]
            psum_rs = psum_pq.tile([BLOC, 1], F32, tag="pq")
            nc.tensor.matmul(psum_rs, lhsT=sel16, rhs=colsum, start=True, stop=True)
            nc.vector.reciprocal(out=rinv_sb, in_=psum_rs)
            # broadcast 1/rowsum back to the 16 partitions of each batch
            psum_ri = psum_pq.tile([128, 1], F32, tag="pq")
            nc.tensor.matmul(psum_ri, lhsT=sel16T, rhs=rinv_sb, start=True, stop=True)
            nc.vector.tensor_scalar_mul(out=work2, in0=work2, scalar1=psum_ri)
            nc.sync.dma_start(
                out=out[:, :].rearrange("b (q f) -> (b q) f", f=PF), in_=work2
            )

    # Run the Bacc lowering passes (move_matmul_waits_to_ldweights,
    # generate_event_semaphores, alloc_regs, ...).
    nc.finalize()
    return nc


_CACHE: dict = {}


def _get_nc(NT: int) -> bass.Bass:
    if NT not in _CACHE:
        _CACHE[NT] = build_nc(NT)
    return _CACHE[NT]


def make_in_maps(query, processed_memory, mask, Wq, v):
    query = np.asarray(query, dtype=np.float32)
    pm = np.asarray(processed_memory, dtype=np.float32)
    mask_b = np.asarray(mask).astype(bool)
    Wq = np.asarray(Wq, dtype=np.float32)
    v = np.asarray(v, dtype=np.float32)

    idx_all = []
    n_all = []
    for gb in range(B):
        idx = np.flatnonzero(~mask_b[gb])
        idx_all.append(idx)
        n_all.append(len(idx))
    n_max = max(n_all)
    NT = max(NT_DEFAULT, ((n_max + 127) // 128) * 128)
    PF = NT // 16

    WqT = np.ascontiguousarray(Wq.T).astype(np.float16)  # (QD, AD)
    zv = np.zeros((128, DB, 255), dtype=np.float16)
    for db in range(DB):
        zv[:, db, 127] = v[db * 128 : (db + 1) * 128]
    sel16 = np.zeros((128, BLOC), dtype=np.float32)
    for b in range(BLOC):
        sel16[b * 16 : (b + 1) * 16, b] = 1.0
    sel16T = np.ascontiguousarray(sel16.T)

    in_maps = []
    for i in range(NCORES):
        sl = slice(i * BLOC, (i + 1) * BLOC)
        pm_core = np.empty((BLOC, 128, DB, NT), dtype=np.float16)
        valid2 = np.zeros((128, PF), dtype=np.float32)
        for b in range(BLOC):
            gb = i * BLOC + b
            idx = idx_all[gb]
            n = n_all[gb]
            if n == 0:
                idx_pad = np.zeros(NT, dtype=np.int64)
            elif n < NT:
                idx_pad = np.concatenate(
                    [idx, np.full(NT - n, idx[-1], dtype=idx.dtype)]
                )
            else:
                idx_pad = idx
            # [NT, AD] -> [AD, NT] -> [128, DB, NT]
            pmt = pm[gb][idx_pad].T.astype(np.float16)
            pm_core[b] = pmt.reshape(DB, 128, NT).transpose(1, 0, 2)
            row = np.zeros(NT, dtype=np.float32)
            row[:n] = 1.0
            valid2[b * 16 : (b + 1) * 16, :] = row.reshape(16, PF)
        in_maps.append(
            {
                "pm_c": pm_core,
                "qT": np.ascontiguousarray(
                    query[sl]
                    .T.reshape(KB, 128, BLOC)
                    .transpose(1, 0, 2)
                    .reshape(128, KB * BLOC)
                ).astype(np.float16),
                "WqT": WqT,
                "zv": zv,
                "valid2": valid2,
                "sel16": sel16,
                "sel16T": sel16T,
            }
        )
    return in_maps, idx_all, n_all, NT


def run_spmd(in_maps, NT=NT_DEFAULT, **kwargs):
    return run_bass_kernel_spmd(_get_nc(NT), in_maps, list(range(NCORES)), **kwargs)


def kernel(query, processed_memory, mask, Wq, v) -> np.ndarray:
    in_maps, idx_all, n_all, NT = make_in_maps(query, processed_memory, mask, Wq, v)
    res = run_spmd(in_maps, NT=NT)
    out_full = np.zeros((B, T), dtype=np.float32)
    for i in range(NCORES):
        oc = res.results[i]["out"]
        for b in range(BLOC):
            gb = i * BLOC + b
            n = n_all[gb]
            if n == 0:
                # reference: all energies equal (-1e30) -> uniform softmax
                out_full[gb, :] = 1.0 / T
            else:
                out_full[gb, idx_all[gb]] = oc[b, :n]
    return out_full
